# revision 7
# baseline (speedup 1.0000x reference)
"""AdaAttN Trainium2 kernel — 8-core SPMD, data-parallel over (batch, query-half).

Each core handles one (batch b, query half): 2048 of the 4096 query positions.
All matmuls are SINGLE-PASS fp32r (fp32 data in the PE's full-rate mode:
1 cycle/row when the moving dim >= 256 — measured 77.8us vs 74.6us bf16 for
256 chained matmuls, absmax err 1.4e-4 vs 2.1e-3 bf16). This replaces the
previous 3-pass bf16-split scheme at ~1/3 the Tensor-engine cycles with
better accuracy.

  F  = f_w @ content_key[b][:, q]        [ck, q]   f32r
  G  = g_w @ style_key[b]                [ck, k]   f32r   (8 MiB, SBUF)
  HT = (h_w @ style[b]).T                [k, c]    f32r   (8 MiB, SBUF)
  S^T[k, q] = G.T @ F                    f32r
  P  = exp(S^T - 120)  (f32, kept per query block: 32 x [128,256])
  l  = sum_k P (DVE accumulate + ones-matmul partition reduce)
  mean  = HT.T @ P     (f32r, PSUM-accumulated over all 32 key tiles)
  h2    = HT^2 (f32, exact squares)  ->  second = h2.T @ P  (f32r)
  out = sqrt(relu(sec/l - (mean/l)^2)) * mvnorm(content) + mean/l + h_b

Query block QB=256 so P (f32) fits SBUF next to G and HT. Per block:
pass A accumulates mean in 4 PSUM banks while the S^T ring uses 3 more;
after l is known, pass B accumulates the second moment in the same 4 banks
(sequential reuse). Variance is computed from consistent quantities (same P
bits in mean/sec/l matmuls; h2 squared exactly in f32) so the
sqrt(sec - mean^2) cancellation does not amplify independent rounding.
h_b is folded into the final add (variance is shift-invariant); f_b/g_b are
added at F/G PSUM evacuation via the ACT bias port.
"""

import contextlib
import ctypes
import sys
import types

import numpy as np

import concourse.bass as bass
import concourse.mybir as mybir
from concourse import bacc
from concourse.bass import ts
from concourse.bass_utils import run_bass_kernel_spmd
from concourse.tile import TileContext

F32 = mybir.dt.float32
F32R = mybir.dt.float32r
AF = mybir.ActivationFunctionType
ALU = mybir.AluOpType

B, C, HW = 4, 512, 4096  # batch, channels (=key planes), spatial
Q = 2048                 # queries per core (half a batch)
QB = 256                 # query block
NBLK = Q // QB           # 8
CC = C // 128            # 4 channel chunks
NKT = HW // 128          # 32 key tiles
SHIFT = 120.0
EPS = 1e-5


# --------------------------------------------------------------------------
# antenv.axon_hooks shim: this image's antenv lacks axon_hooks, which makes
# run_bass_kernel_spmd(trace=True) raise ImportError instead of degrading.
# Provide the hook via the same ctypes protocol trn_boot uses.
def _install_axon_hooks_shim():
    if "antenv.axon_hooks" in sys.modules:
        return
    try:
        import antenv.axon_hooks  # noqa: F401

        return
    except ImportError:
        pass

    def _make_hook(so_path):
        try:
            lib = ctypes.CDLL(so_path)
        except OSError:
            return None
        if not hasattr(lib, "axon_start_nrt_profile"):
            return None
        lib.axon_start_nrt_profile.argtypes = [
            ctypes.POINTER(ctypes.c_int64),
            ctypes.c_size_t,
        ]
        lib.axon_start_nrt_profile.restype = ctypes.c_int64
        lib.axon_stop_nrt_profile.argtypes = [ctypes.c_char_p]
        lib.axon_stop_nrt_profile.restype = ctypes.c_int64

        @contextlib.contextmanager
        def _hook(output_dir, device_ids):
            import jax

            jax.devices()
            if device_ids:
                ids = (ctypes.c_int64 * len(device_ids))(*device_ids)
                rc = lib.axon_start_nrt_profile(ids, len(device_ids))
            else:
                rc = lib.axon_start_nrt_profile(None, 0)
            if rc != 0:
                raise RuntimeError(f"axon_start_nrt_profile rc={rc}")
            try:
                yield
            finally:
                n = lib.axon_stop_nrt_profile(str(output_dir).encode())
                if n < 0:
                    raise RuntimeError(f"axon_stop_nrt_profile rc={n}")

        return _hook

    mod = types.ModuleType("antenv.axon_hooks")
    box = [_make_hook("/opt/axon/libaxon_pjrt.so")]
    mod.get_axon_ntff_profile_hook = lambda: box[0]
    mod.set_axon_ntff_profile_hook = lambda h: box.__setitem__(0, h)
    sys.modules["antenv.axon_hooks"] = mod


def _build():
    nc = bacc.Bacc("TRN2", target_bir_lowering=False, debug=False)

    ckq = nc.declare_dram_parameter("ckq", [C, Q], F32, isOutput=False)
    sk = nc.declare_dram_parameter("sk", [C, HW], F32, isOutput=False)
    st = nc.declare_dram_parameter("st", [C, HW], F32, isOutput=False)
    ct = nc.declare_dram_parameter("ct", [C, HW], F32, isOutput=False)
    ctq = nc.declare_dram_parameter("ctq", [C, Q], F32, isOutput=False)
    fwT = nc.declare_dram_parameter("fwT", [C, C], F32, isOutput=False)
    gwT = nc.declare_dram_parameter("gwT", [C, C], F32, isOutput=False)
    hwT = nc.declare_dram_parameter("hwT", [C, C], F32, isOutput=False)
    fb = nc.declare_dram_parameter("fb", [C, 1], F32, isOutput=False)
    gb = nc.declare_dram_parameter("gb", [C, 1], F32, isOutput=False)
    hb = nc.declare_dram_parameter("hb", [C, 1], F32, isOutput=False)
    out = nc.declare_dram_parameter("out", [C, Q], F32, isOutput=True)

    # [512, M] dram -> [128, 4, M] (partition = channel-within-chunk)
    def chunked(ap):
        return ap.rearrange("(a p) m -> p a m", p=128)

    def r(ap):
        return ap.bitcast(F32R)

    with TileContext(nc) as tc:
        with (
            tc.tile_pool(name="const", bufs=1) as const,
            tc.tile_pool(name="stage", bufs=2) as stage,
            tc.tile_pool(name="wslot", bufs=1) as wslot,
            tc.tile_pool(name="big", bufs=1) as big,
            tc.tile_pool(name="pslab", bufs=32) as pslab,
            tc.tile_pool(name="work", bufs=2) as work,
            tc.tile_pool(name="scratch", bufs=1) as scratch,
            tc.tile_pool(name="psacc", bufs=4, space="PSUM") as psacc,
            tc.tile_pool(name="psmm", bufs=3, space="PSUM") as psmm,
        ):
            # ---------------- constants ----------------
            fwT_sb = const.tile([128, CC, C], F32R)
            nc.sync.dma_start(out=fwT_sb, in_=chunked(fwT.ap()).bitcast(F32R))
            fb_sb = const.tile([128, CC, 1], F32)
            gb_sb = const.tile([128, CC, 1], F32)
            hb_sb = const.tile([128, CC, 1], F32)
            nc.sync.dma_start(out=fb_sb, in_=chunked(fb.ap()))
            nc.sync.dma_start(out=gb_sb, in_=chunked(gb.ap()))
            nc.sync.dma_start(out=hb_sb, in_=chunked(hb.ap()))
            negshift = const.tile([128, 1], F32)
            nc.vector.memset(negshift, -SHIFT)
            ones_f = const.tile([128, 1], F32)
            nc.vector.memset(ones_f, 1.0)
            cmean = const.tile([128, CC, 1], F32)
            crstd2 = const.tile([128, CC, 1], F32)

            # ---------------- G = g_w @ style_key (f32r) ----------------
            G = big.tile([128, CC, HW], F32R)
            sk_ch = chunked(sk.ap())
            gwT_sb = wslot.tile([128, CC, C], F32R, tag="w", name="gwT_sb")
            nc.sync.dma_start(out=gwT_sb, in_=chunked(gwT.ap()).bitcast(F32R))
            for w in range(HW // 512):
                skA = stage.tile([128, 2, 512], F32R, tag="stage", name=f"skA{w}")
                nc.sync.dma_start(out=skA, in_=sk_ch[:, 0:2, ts(w, 512)].bitcast(F32R))
                skB = stage.tile([128, 2, 512], F32R, tag="stage", name=f"skB{w}")
                nc.sync.dma_start(out=skB, in_=sk_ch[:, 2:4, ts(w, 512)].bitcast(F32R))
                for sub in range(2):
                    for co in range(CC):
                        gps = psmm.tile([128, 256], F32, tag="mm")
                        for ci in range(CC):
                            src = skA if ci < 2 else skB
                            nc.tensor.matmul(
                                gps,
                                gwT_sb[:, ci, ts(co, 128)],
                                src[:, ci % 2, ts(sub, 256)],
                                start=(ci == 0),
                                stop=(ci == CC - 1),
                            )
                        nc.vector.tensor_scalar_add(
                            G[:, co, w * 512 + sub * 256 : w * 512 + (sub + 1) * 256],
                            gps,
                            gb_sb[:, co, :],
                        )

            # ---------------- HT[k, c] = (h_w @ style).T (f32r) ----------------
            HTF = big.tile([128, NKT, C], F32R)
            st_ch = chunked(st.ap())
            hwT_sb = wslot.tile([128, CC, C], F32R, tag="w", name="hwT_sb")
            nc.sync.dma_start(out=hwT_sb, in_=chunked(hwT.ap()).bitcast(F32R))
            for w in range(HW // 512):
                stA = stage.tile([128, 2, 512], F32R, tag="stage", name=f"stA{w}")
                nc.sync.dma_start(out=stA, in_=st_ch[:, 0:2, ts(w, 512)].bitcast(F32R))
                stB = stage.tile([128, 2, 512], F32R, tag="stage", name=f"stB{w}")
                nc.sync.dma_start(out=stB, in_=st_ch[:, 2:4, ts(w, 512)].bitcast(F32R))
                for sub in range(4):
                    kt = w * 4 + sub
                    hps = psmm.tile([128, 512], F32, tag="mm")
                    for ci in range(CC):
                        src = stA if ci < 2 else stB
                        nc.tensor.matmul(
                            hps,
                            src[:, ci % 2, ts(sub, 128)],
                            hwT_sb[:, ci, :],
                            start=(ci == 0),
                            stop=(ci == CC - 1),
                        )
                    nc.scalar.activation(
                        HTF[:, kt, :], hps, AF.Copy, bias=0.0, scale=1.0
                    )

            # ---------------- content mvn stats (streamed during block 0) ----
            ct_ch = chunked(ct.ap())
            stats_all = scratch.tile([128, 4, 8, 6], F32, tag="bnstats")

            def emit_stats_piece(i):
                cc, quart = i // 4, i % 4
                ctp = stage.tile([128, 2, 512], F32, tag="stage", name=f"ctp{i}")
                nc.sync.dma_start(
                    out=ctp,
                    in_=ct_ch[:, cc, ts(quart, 1024)].rearrange(
                        "p (a m) -> p a m", a=2
                    ),
                )
                for g in range(2):
                    nc.vector.bn_stats(
                        out=stats_all[:, cc, quart * 2 + g, :], in_=ctp[:, g, :]
                    )

            def emit_stats_tail():
                for cc in range(CC):
                    mv = scratch.tile([128, 2], F32, tag="bnmv")
                    nc.vector.bn_aggr(
                        out=mv,
                        in_=stats_all[:, cc, :, :].rearrange("p a b -> p (a b)"),
                    )
                    nc.vector.tensor_copy(cmean[:, cc, :], mv[:, 0:1])
                    tv = scratch.tile([128, 1], F32, tag="bntv")
                    nc.vector.tensor_scalar(
                        out=tv,
                        in0=mv[:, 1:2],
                        scalar1=float(HW) / float(HW - 1),
                        scalar2=EPS,
                        op0=ALU.mult,
                        op1=ALU.add,
                    )
                    sq = scratch.tile([128, 1], F32, tag="bnsq")
                    nc.scalar.activation(sq, tv, AF.Sqrt, bias=0.0, scale=1.0)
                    rs = scratch.tile([128, 1], F32, tag="bnrs")
                    nc.vector.reciprocal(rs, sq)
                    nc.vector.tensor_mul(crstd2[:, cc, :], rs, rs)

            # ---------------- main loop over query blocks ----------------
            ckq_ch = chunked(ckq.ap())
            ctq_ch = chunked(ctq.ap())
            out_ch = chunked(out.ap())

            for blk in range(NBLK):
                # F block (f32r), evac with f_b bias
                ckq_t = stage.tile([128, CC, QB], F32R, tag="stage", name=f"ckq{blk}")
                nc.sync.dma_start(
                    out=ckq_t, in_=ckq_ch[:, :, ts(blk, QB)].bitcast(F32R)
                )
                Fb = work.tile([128, CC, QB], F32R, tag="f", bufs=1, name=f"Fb{blk}")
                for co in range(CC):
                    fps = psmm.tile([128, QB], F32, tag="mm")
                    for ci in range(CC):
                        nc.tensor.matmul(
                            fps,
                            fwT_sb[:, ci, ts(co, 128)],
                            ckq_t[:, ci, :],
                            start=(ci == 0),
                            stop=(ci == CC - 1),
                        )
                    nc.vector.tensor_scalar_add(
                        Fb[:, co, :], fps, fb_sb[:, co, :]
                    )

                mean_ps = [
                    psacc.tile([128, QB], F32, tag="acc", name=f"mean{i}")
                    for i in range(CC)
                ]
                l_part = work.tile([128, QB], F32, tag="l", bufs=1)
                pts = {}

                def emit_st(kt):
                    sps = psmm.tile([128, QB], F32, tag="mm", name=f"sps{kt}")
                    for ci in range(CC):
                        nc.tensor.matmul(
                            sps,
                            G[:, ci, ts(kt, 128)],
                            Fb[:, ci, :],
                            start=(ci == 0),
                            stop=(ci == CC - 1),
                        )
                    pt = pslab.tile([128, QB], F32R, tag="P", name=f"pt{kt}")
                    nc.scalar.activation(pt, sps, AF.Exp, bias=negshift, scale=1.0)
                    ptf = pt.bitcast(F32)
                    if kt == 0:
                        nc.vector.tensor_copy(l_part, ptf)
                    else:
                        nc.vector.tensor_add(l_part, l_part, ptf)
                    pts[kt] = pt

                def emit_mean(kt):
                    pt = pts[kt]
                    for cc in range(CC):
                        nc.tensor.matmul(
                            mean_ps[cc],
                            HTF[:, kt, ts(cc, 128)],
                            pt,
                            start=(kt == 0),
                            stop=(kt == NKT - 1),
                        )

                emit_st(0)
                if blk == 0:
                    emit_stats_piece(0)
                for kt in range(1, NKT):
                    emit_st(kt)
                    if blk == 0 and kt < 16:
                        emit_stats_piece(kt)
                    emit_mean(kt - 1)
                if blk == 0:
                    emit_stats_tail()
                emit_mean(NKT - 1)

                # l partition-reduce, reciprocal, broadcast
                l_ps = psmm.tile([1, QB], F32, tag="mm", name="lps")
                nc.tensor.matmul(l_ps, ones_f, l_part, start=True, stop=True)
                rinv = scratch.tile([1, QB], F32, tag="ptmp")
                nc.vector.reciprocal(rinv, l_ps)
                rbc = work.tile([128, QB], F32, tag="rbc", bufs=1)
                nc.gpsimd.partition_broadcast(rbc, rinv[:1, :])

                # evacuate mean accumulators (ACT) to free the 4 banks
                macc = work.tile([128, CC, QB], F32, tag="macc", bufs=1)
                for cc in range(CC):
                    nc.scalar.activation(
                        macc[:, cc, :], mean_ps[cc], AF.Copy, bias=0.0, scale=1.0
                    )

                # ---- pass B: second moment (h2 exact squares, f32r) ----
                sec_ps = [
                    psacc.tile([128, QB], F32, tag="acc", name=f"sec{i}")
                    for i in range(CC)
                ]

                def emit_h2(kt):
                    h2 = work.tile([128, C], F32R, tag="h2", name=f"h2_{kt}")
                    htf = HTF[:, kt, :].bitcast(F32)
                    if kt % 2 == 0:
                        nc.scalar.activation(h2, htf, AF.Square, bias=0.0, scale=1.0)
                    else:
                        nc.vector.tensor_mul(h2, htf, htf)
                    return h2

                h2_cur = emit_h2(0)
                for kt in range(NKT):
                    h2_next = emit_h2(kt + 1) if kt + 1 < NKT else None
                    pt = pts.pop(kt)
                    for cc in range(CC):
                        nc.tensor.matmul(
                            sec_ps[cc],
                            h2_cur[:, ts(cc, 128)],
                            pt,
                            start=(kt == 0),
                            stop=(kt == NKT - 1),
                        )
                    h2_cur = h2_next

                # ---- post: variance, std, assemble output ----
                ctq_t = stage.tile([128, CC, QB], F32, tag="stage", name=f"ctq{blk}")
                nc.sync.dma_start(out=ctq_t, in_=ctq_ch[:, :, ts(blk, QB)])
                for cc in range(CC):
                    mnp = work.tile([128, QB], F32, tag="mnp", bufs=1)
                    nc.vector.tensor_mul(mnp, macc[:, cc, :], rbc)
                    e2 = work.tile([128, QB], F32, tag="e2", bufs=1)
                    nc.vector.tensor_mul(e2, sec_ps[cc], rbc)
                    msq = work.tile([128, QB], F32, tag="msq", bufs=1)
                    nc.scalar.activation(msq, mnp, AF.Square, bias=0.0, scale=1.0)
                    var = work.tile([128, QB], F32, tag="var", bufs=1)
                    nc.vector.tensor_sub(var, e2, msq)
                    # vmx = relu(var) reuses the e2 slot; stdt = sqrt(vmx)/cstd
                    # reuses msq; o1 reuses var — keeps the tag count low.
                    nc.scalar.activation(e2, var, AF.Relu, bias=0.0, scale=1.0)
                    nc.scalar.activation(
                        msq, e2, AF.Sqrt, bias=0.0, scale=crstd2[:, cc, :]
                    )
                    nc.vector.scalar_tensor_tensor(
                        out=var,
                        in0=ctq_t[:, cc, :],
                        scalar=cmean[:, cc, :],
                        in1=msq,
                        op0=ALU.subtract,
                        op1=ALU.mult,
                    )
                    out_sb = work.tile([128, QB], F32, tag="outb", bufs=1)
                    nc.vector.scalar_tensor_tensor(
                        out=out_sb,
                        in0=mnp,
                        scalar=hb_sb[:, cc, :],
                        in1=var,
                        op0=ALU.add,
                        op1=ALU.add,
                    )
                    nc.sync.dma_start(out=out_ch[:, cc, ts(blk, QB)], in_=out_sb)

    nc.compile()
    return nc


_NC_CACHE = []


def kernel(content, style, content_key, style_key, f_w, f_b, g_w, g_b, h_w, h_b):
    _install_axon_hooks_shim()
    if not _NC_CACHE:
        _NC_CACHE.append(_build())
    nc = _NC_CACHE[0]

    c32 = lambda a: np.ascontiguousarray(np.asarray(a), dtype=np.float32)

    fwT = c32(np.asarray(f_w).T)
    gwT = c32(np.asarray(g_w).T)
    hwT = c32(np.asarray(h_w).T)
    fbr = c32(np.asarray(f_b).reshape(C, 1))
    gbr = c32(np.asarray(g_b).reshape(C, 1))
    hbr = c32(np.asarray(h_b).reshape(C, 1))

    in_maps = []
    for core in range(8):
        b, h = core // 2, core % 2
        qsl = slice(h * Q, (h + 1) * Q)
        in_maps.append(
            {
                "ckq": c32(np.asarray(content_key[b]).reshape(C, HW)[:, qsl]),
                "sk": c32(np.asarray(style_key[b]).reshape(C, HW)),
                "st": c32(np.asarray(style[b]).reshape(C, HW)),
                "ct": c32(np.asarray(content[b]).reshape(C, HW)),
                "ctq": c32(np.asarray(content[b]).reshape(C, HW)[:, qsl]),
                "fwT": fwT,
                "gwT": gwT,
                "hwT": hwT,
                "fb": fbr,
                "gb": gbr,
                "hb": hbr,
            }
        )

    try:
        res = run_bass_kernel_spmd(nc, in_maps, core_ids=list(range(8)), trace=True)
    except Exception:
        res = run_bass_kernel_spmd(nc, in_maps, core_ids=list(range(8)), trace=False)
    kernel.last_exec_time_ns = res.exec_time_ns

    full = np.empty((B, C, HW), dtype=np.float32)
    for core in range(8):
        b, h = core // 2, core % 2
        full[b][:, h * Q : (h + 1) * Q] = res.results[core]["out"]
    return full.reshape(B, C, 64, 64)


kernel.last_exec_time_ns = None


# revision 8
# speedup vs baseline: 1.0045x; 1.0045x over previous
"""AdaAttN Trainium2 kernel — 8-core SPMD, data-parallel over (batch, query-half).

Each core handles one (batch b, query half): 2048 of the 4096 query positions,
processed in 4 query blocks of QB=512 so every matmul has a 512-wide moving
operand (the 4-byte f32r LDWEIGHTS (~203ns) only hides behind matmuls with
free dim >= 512; at 256 the pipeline is weight-load-bound).

Numerics (validated with a bit-faithful CPU simulator; f32r == operands
rounded to 11-bit mantissa, fp16 matmuls exact-in-f32-accumulate):
  F  = f_w @ content_key[b][:, q]     f32r matmuls -> stored fp16
  G  = g_w @ style_key[b]             f32r -> stored fp16   (32 KiB)
  HT = (h_w @ style[b]).T             f32r -> stored fp16   (32 KiB)
  S^T[k, q] = G.T @ F                 fp16 x fp16 (exact products)
  P  = exp(S^T - 120) stored f32r     (rounding happens at production, so
                                       the l-sum and both PV matmuls see
                                       identical P bits -> consistent stats)
  l  = sum_k P (DVE) + ones-matmul partition reduce (plain fp32)
  vr[kt] = f32r copy of HT[kt]        (fp16 values are f32r-exact, so
  mean = vr^T P / l                    mean is the EXACT weighted mean of vr)
  h2 = vr^2 exactly in f32; h2a = f32r(h2), h2b = f32r(h2 - h2a)
  sec = (h2a + h2b)^T P / l           (~22-bit second moment: kills the
                                       sqrt(sec - mean^2) cancellation noise)
  out = sqrt(relu(sec - mean^2)) * mvnorm(content) + mean + h_b

PSUM: pass A holds mean accumulators (4 banks) + S ring (3); pass B reuses
the same 4 banks for sec (start=True owns a whole bank, so groups never
share). h_b folds into the final add; f_b/g_b are added at PSUM evacuation.
"""

import contextlib
import ctypes
import sys
import types

import numpy as np

import concourse.bass as bass
import concourse.mybir as mybir
from concourse import bacc
from concourse.bass import ts
from concourse.bass_utils import run_bass_kernel_spmd
from concourse.tile import TileContext

F32 = mybir.dt.float32
F32R = mybir.dt.float32r
FP16 = mybir.dt.float16
AF = mybir.ActivationFunctionType
ALU = mybir.AluOpType

B, C, HW = 4, 512, 4096  # batch, channels (=key planes), spatial
Q = 2048                 # queries per core (half a batch)
QB = 512                 # query block
NBLK = Q // QB           # 4
CC = C // 128            # 4 channel chunks
NKT = HW // 128          # 32 key tiles
SHIFT = 120.0
EPS = 1e-5


# --------------------------------------------------------------------------
# antenv.axon_hooks shim: this image's antenv lacks axon_hooks, which makes
# run_bass_kernel_spmd(trace=True) raise ImportError instead of degrading.
# Provide the hook via the same ctypes protocol trn_boot uses.
def _install_axon_hooks_shim():
    if "antenv.axon_hooks" in sys.modules:
        return
    try:
        import antenv.axon_hooks  # noqa: F401

        return
    except ImportError:
        pass

    def _make_hook(so_path):
        try:
            lib = ctypes.CDLL(so_path)
        except OSError:
            return None
        if not hasattr(lib, "axon_start_nrt_profile"):
            return None
        lib.axon_start_nrt_profile.argtypes = [
            ctypes.POINTER(ctypes.c_int64),
            ctypes.c_size_t,
        ]
        lib.axon_start_nrt_profile.restype = ctypes.c_int64
        lib.axon_stop_nrt_profile.argtypes = [ctypes.c_char_p]
        lib.axon_stop_nrt_profile.restype = ctypes.c_int64

        @contextlib.contextmanager
        def _hook(output_dir, device_ids):
            import jax

            jax.devices()
            if device_ids:
                ids = (ctypes.c_int64 * len(device_ids))(*device_ids)
                rc = lib.axon_start_nrt_profile(ids, len(device_ids))
            else:
                rc = lib.axon_start_nrt_profile(None, 0)
            if rc != 0:
                raise RuntimeError(f"axon_start_nrt_profile rc={rc}")
            try:
                yield
            finally:
                n = lib.axon_stop_nrt_profile(str(output_dir).encode())
                if n < 0:
                    raise RuntimeError(f"axon_stop_nrt_profile rc={n}")

        return _hook

    mod = types.ModuleType("antenv.axon_hooks")
    box = [_make_hook("/opt/axon/libaxon_pjrt.so")]
    mod.get_axon_ntff_profile_hook = lambda: box[0]
    mod.set_axon_ntff_profile_hook = lambda h: box.__setitem__(0, h)
    sys.modules["antenv.axon_hooks"] = mod


def _build():
    nc = bacc.Bacc("TRN2", target_bir_lowering=False, debug=False)

    ckq = nc.declare_dram_parameter("ckq", [C, Q], F32, isOutput=False)
    sk = nc.declare_dram_parameter("sk", [C, HW], F32, isOutput=False)
    st = nc.declare_dram_parameter("st", [C, HW], F32, isOutput=False)
    ct = nc.declare_dram_parameter("ct", [C, HW], F32, isOutput=False)
    ctq = nc.declare_dram_parameter("ctq", [C, Q], F32, isOutput=False)
    fwT = nc.declare_dram_parameter("fwT", [C, C], F32, isOutput=False)
    gwT = nc.declare_dram_parameter("gwT", [C, C], F32, isOutput=False)
    hwT = nc.declare_dram_parameter("hwT", [C, C], F32, isOutput=False)
    fb = nc.declare_dram_parameter("fb", [C, 1], F32, isOutput=False)
    gb = nc.declare_dram_parameter("gb", [C, 1], F32, isOutput=False)
    hb = nc.declare_dram_parameter("hb", [C, 1], F32, isOutput=False)
    out = nc.declare_dram_parameter("out", [C, Q], F32, isOutput=True)

    # [512, M] dram -> [128, 4, M] (partition = channel-within-chunk)
    def chunked(ap):
        return ap.rearrange("(a p) m -> p a m", p=128)

    with TileContext(nc) as tc:
        with (
            tc.tile_pool(name="const", bufs=1) as const,
            tc.tile_pool(name="stage", bufs=3) as stage,
            tc.tile_pool(name="wslot", bufs=1) as wslot,
            tc.tile_pool(name="big", bufs=1) as big,
            tc.tile_pool(name="pslab", bufs=32) as pslab,
            tc.tile_pool(name="work", bufs=2) as work,
            tc.tile_pool(name="scratch", bufs=1) as scratch,
            tc.tile_pool(name="psacc", bufs=4, space="PSUM") as psacc,
            tc.tile_pool(name="psmm", bufs=3, space="PSUM") as psmm,
        ):
            # ---------------- constants ----------------
            fwT_sb = const.tile([128, CC, C], F32R)
            nc.sync.dma_start(out=fwT_sb, in_=chunked(fwT.ap()).bitcast(F32R))
            fb_sb = const.tile([128, CC, 1], F32)
            gb_sb = const.tile([128, CC, 1], F32)
            hb_sb = const.tile([128, CC, 1], F32)
            nc.sync.dma_start(out=fb_sb, in_=chunked(fb.ap()))
            nc.sync.dma_start(out=gb_sb, in_=chunked(gb.ap()))
            nc.sync.dma_start(out=hb_sb, in_=chunked(hb.ap()))
            negshift = const.tile([128, 1], F32)
            nc.vector.memset(negshift, -SHIFT)
            ones_f = const.tile([128, 1], F32)
            nc.vector.memset(ones_f, 1.0)
            cmean = const.tile([128, CC, 1], F32)
            crstd2 = const.tile([128, CC, 1], F32)

            # ---------------- G = g_w @ style_key (f32r -> fp16) ----------
            G = big.tile([128, CC, HW], FP16)
            sk_ch = chunked(sk.ap())
            gwT_sb = wslot.tile([128, CC, C], F32R, tag="w", name="gwT_sb")
            nc.sync.dma_start(out=gwT_sb, in_=chunked(gwT.ap()).bitcast(F32R))
            for w in range(HW // 512):
                skA = stage.tile([128, 2, 512], F32R, tag="stage", name=f"skA{w}")
                nc.sync.dma_start(
                    out=skA, in_=sk_ch[:, 0:2, ts(w, 512)].bitcast(F32R)
                )
                skB = stage.tile([128, 2, 512], F32R, tag="stage", name=f"skB{w}")
                nc.sync.dma_start(
                    out=skB, in_=sk_ch[:, 2:4, ts(w, 512)].bitcast(F32R)
                )
                for co in range(CC):
                    gps = psmm.tile([128, 512], F32, tag="mm")
                    for ci in range(CC):
                        src = skA if ci < 2 else skB
                        nc.tensor.matmul(
                            gps,
                            gwT_sb[:, ci, ts(co, 128)],
                            src[:, ci % 2, :],
                            start=(ci == 0),
                            stop=(ci == CC - 1),
                        )
                    nc.vector.tensor_scalar_add(
                        G[:, co, ts(w, 512)], gps, gb_sb[:, co, :]
                    )

            # ---------------- HT[k, c] = (h_w @ style).T (f32r -> fp16) ----
            HTF = big.tile([128, NKT, C], FP16)
            st_ch = chunked(st.ap())
            hwT_sb = wslot.tile([128, CC, C], F32R, tag="w", name="hwT_sb")
            nc.sync.dma_start(out=hwT_sb, in_=chunked(hwT.ap()).bitcast(F32R))
            for w in range(HW // 512):
                stA = stage.tile([128, 2, 512], F32R, tag="stage", name=f"stA{w}")
                nc.sync.dma_start(
                    out=stA, in_=st_ch[:, 0:2, ts(w, 512)].bitcast(F32R)
                )
                stB = stage.tile([128, 2, 512], F32R, tag="stage", name=f"stB{w}")
                nc.sync.dma_start(
                    out=stB, in_=st_ch[:, 2:4, ts(w, 512)].bitcast(F32R)
                )
                for sub in range(4):
                    kt = w * 4 + sub
                    hps = psmm.tile([128, 512], F32, tag="mm")
                    for ci in range(CC):
                        src = stA if ci < 2 else stB
                        nc.tensor.matmul(
                            hps,
                            src[:, ci % 2, ts(sub, 128)],
                            hwT_sb[:, ci, :],
                            start=(ci == 0),
                            stop=(ci == CC - 1),
                        )
                    nc.scalar.activation(
                        HTF[:, kt, :], hps, AF.Copy, bias=0.0, scale=1.0
                    )

            # ---------------- content mvn stats (streamed during block 0) --
            ct_ch = chunked(ct.ap())
            stats_all = scratch.tile([128, 4, 8, 6], F32, tag="bnstats")

            def emit_stats_piece(i):
                cc, quart = i // 4, i % 4
                ctp = stage.tile([128, 2, 512], F32, tag="stage", name=f"ctp{i}")
                nc.sync.dma_start(
                    out=ctp,
                    in_=ct_ch[:, cc, ts(quart, 1024)].rearrange(
                        "p (a m) -> p a m", a=2
                    ),
                )
                for g in range(2):
                    nc.vector.bn_stats(
                        out=stats_all[:, cc, quart * 2 + g, :], in_=ctp[:, g, :]
                    )

            def emit_stats_tail():
                for cc in range(CC):
                    mv = scratch.tile([128, 2], F32, tag="bnmv")
                    nc.vector.bn_aggr(
                        out=mv,
                        in_=stats_all[:, cc, :, :].rearrange("p a b -> p (a b)"),
                    )
                    nc.vector.tensor_copy(cmean[:, cc, :], mv[:, 0:1])
                    tv = scratch.tile([128, 1], F32, tag="bntv")
                    nc.vector.tensor_scalar(
                        out=tv,
                        in0=mv[:, 1:2],
                        scalar1=float(HW) / float(HW - 1),
                        scalar2=EPS,
                        op0=ALU.mult,
                        op1=ALU.add,
                    )
                    sq = scratch.tile([128, 1], F32, tag="bnsq")
                    nc.scalar.activation(sq, tv, AF.Sqrt, bias=0.0, scale=1.0)
                    rs = scratch.tile([128, 1], F32, tag="bnrs")
                    nc.vector.reciprocal(rs, sq)
                    nc.vector.tensor_mul(crstd2[:, cc, :], rs, rs)

            # ---------------- main loop over query blocks ----------------
            ckq_ch = chunked(ckq.ap())
            ctq_ch = chunked(ctq.ap())
            out_ch = chunked(out.ap())

            # block-0 ckq prefetch
            ckq_pair = {}
            ckq_pair[0] = (
                stage.tile([128, 2, QB], F32R, tag="stage", name="ckqA0"),
                stage.tile([128, 2, QB], F32R, tag="stage", name="ckqB0"),
            )
            nc.sync.dma_start(
                out=ckq_pair[0][0], in_=ckq_ch[:, 0:2, 0:QB].bitcast(F32R)
            )
            nc.sync.dma_start(
                out=ckq_pair[0][1], in_=ckq_ch[:, 2:4, 0:QB].bitcast(F32R)
            )

            for blk in range(NBLK):
                ckqA, ckqB = ckq_pair.pop(blk)
                Fb = work.tile([128, CC, QB], FP16, tag="f", bufs=1, name=f"Fb{blk}")
                for co in range(CC):
                    fps = psmm.tile([128, QB], F32, tag="mm")
                    for ci in range(CC):
                        src = ckqA if ci < 2 else ckqB
                        nc.tensor.matmul(
                            fps,
                            fwT_sb[:, ci, ts(co, 128)],
                            src[:, ci % 2, :],
                            start=(ci == 0),
                            stop=(ci == CC - 1),
                        )
                    nc.vector.tensor_scalar_add(
                        Fb[:, co, :], fps, fb_sb[:, co, :]
                    )

                mean_ps = [
                    psacc.tile([128, QB], F32, tag="acc", name=f"mean{i}")
                    for i in range(CC)
                ]
                l_part = work.tile([128, QB], F32, tag="l", bufs=1)
                pts = {}

                def emit_st(kt):
                    sps = psmm.tile([128, QB], F32, tag="mm", name=f"sps{kt}")
                    for ci in range(CC):
                        nc.tensor.matmul(
                            sps,
                            G[:, ci, ts(kt, 128)],
                            Fb[:, ci, :],
                            start=(ci == 0),
                            stop=(ci == CC - 1),
                        )
                    pt = pslab.tile([128, QB], F32R, tag="P", name=f"pt{kt}")
                    nc.scalar.activation(pt, sps, AF.Exp, bias=negshift, scale=1.0)
                    ptf = pt.bitcast(F32)
                    if kt == 0:
                        nc.vector.tensor_copy(l_part, ptf)
                    else:
                        nc.vector.tensor_add(l_part, l_part, ptf)
                    pts[kt] = pt

                def emit_mean(kt):
                    vr = work.tile([128, C], F32R, tag="vr", bufs=3, name=f"vr{kt}")
                    nc.scalar.activation(
                        vr, HTF[:, kt, :], AF.Copy, bias=0.0, scale=1.0
                    )
                    pt = pts[kt]
                    for cc in range(CC):
                        nc.tensor.matmul(
                            mean_ps[cc],
                            vr[:, ts(cc, 128)],
                            pt,
                            start=(kt == 0),
                            stop=(kt == NKT - 1),
                        )

                emit_st(0)
                if blk == 0:
                    emit_stats_piece(0)
                for kt in range(1, NKT):
                    emit_st(kt)
                    if blk == 0 and kt < 16:
                        emit_stats_piece(kt)
                    emit_mean(kt - 1)
                if blk == 0:
                    emit_stats_tail()
                emit_mean(NKT - 1)

                # prefetch next block's ckq while pass B runs
                if blk + 1 < NBLK:
                    nA = stage.tile(
                        [128, 2, QB], F32R, tag="stage", name=f"ckqA{blk+1}"
                    )
                    nc.sync.dma_start(
                        out=nA,
                        in_=ckq_ch[:, 0:2, ts(blk + 1, QB)].bitcast(F32R),
                    )
                    nB = stage.tile(
                        [128, 2, QB], F32R, tag="stage", name=f"ckqB{blk+1}"
                    )
                    nc.sync.dma_start(
                        out=nB,
                        in_=ckq_ch[:, 2:4, ts(blk + 1, QB)].bitcast(F32R),
                    )
                    ckq_pair[blk + 1] = (nA, nB)

                # l partition-reduce (plain fp32 matmul), reciprocal, bcast
                l_ps = psmm.tile([1, QB], F32, tag="mm", name="lps")
                nc.tensor.matmul(l_ps, ones_f, l_part, start=True, stop=True)
                rinv = scratch.tile([1, QB], F32, tag="ptmp")
                nc.vector.reciprocal(rinv, l_ps)
                rbc = work.tile([128, QB], F32, tag="rbc", bufs=1)
                nc.gpsimd.partition_broadcast(rbc, rinv[:1, :])

                # evacuate mean accumulators (ACT) to free the 4 banks
                macc = work.tile([128, CC, QB], F32, tag="macc", bufs=1)
                for cc in range(CC):
                    nc.scalar.activation(
                        macc[:, cc, :], mean_ps[cc], AF.Copy, bias=0.0, scale=1.0
                    )

                # ---- pass B: second moment via exact-squared split pair ----
                sec_ps = [
                    psacc.tile([128, QB], F32, tag="acc", name=f"sec{i}")
                    for i in range(CC)
                ]

                def emit_h2(kt):
                    h2f = work.tile([128, C], F32, tag="h2f", name=f"h2f{kt}")
                    nc.scalar.activation(
                        h2f, HTF[:, kt, :], AF.Square, bias=0.0, scale=1.0
                    )
                    h2a = work.tile([128, C], F32R, tag="h2a", name=f"h2a{kt}")
                    nc.vector.tensor_copy(h2a, h2f)
                    h2b = work.tile([128, C], F32R, tag="h2b", name=f"h2b{kt}")
                    nc.vector.tensor_sub(h2b, h2f, h2a.bitcast(F32))
                    return h2a, h2b

                h2_cur = emit_h2(0)
                for kt in range(NKT):
                    h2_next = emit_h2(kt + 1) if kt + 1 < NKT else None
                    pt = pts.pop(kt)
                    h2a, h2b = h2_cur
                    for cc in range(CC):
                        nc.tensor.matmul(
                            sec_ps[cc],
                            h2a[:, ts(cc, 128)],
                            pt,
                            start=(kt == 0),
                            stop=False,
                        )
                        nc.tensor.matmul(
                            sec_ps[cc],
                            h2b[:, ts(cc, 128)],
                            pt,
                            start=False,
                            stop=(kt == NKT - 1),
                        )
                    h2_cur = h2_next

                # ---- post: variance, std, assemble output ----
                ctqA = stage.tile([128, 2, QB], F32, tag="stage", name=f"ctqA{blk}")
                nc.sync.dma_start(out=ctqA, in_=ctq_ch[:, 0:2, ts(blk, QB)])
                ctqB = stage.tile([128, 2, QB], F32, tag="stage", name=f"ctqB{blk}")
                nc.sync.dma_start(out=ctqB, in_=ctq_ch[:, 2:4, ts(blk, QB)])
                for cc in range(CC):
                    ctq_t = (ctqA if cc < 2 else ctqB)[:, cc % 2, :]
                    mnp = work.tile([128, QB], F32, tag="mnp", bufs=1)
                    nc.vector.tensor_mul(mnp, macc[:, cc, :], rbc)
                    e2 = work.tile([128, QB], F32, tag="e2", bufs=1)
                    nc.vector.tensor_mul(e2, sec_ps[cc], rbc)
                    msq = work.tile([128, QB], F32, tag="msq", bufs=1)
                    nc.scalar.activation(msq, mnp, AF.Square, bias=0.0, scale=1.0)
                    var = work.tile([128, QB], F32, tag="var", bufs=1)
                    nc.vector.tensor_sub(var, e2, msq)
                    # vmx = relu(var) reuses e2; stdt = sqrt(vmx)/cstd reuses msq
                    nc.scalar.activation(e2, var, AF.Relu, bias=0.0, scale=1.0)
                    nc.scalar.activation(
                        msq, e2, AF.Sqrt, bias=0.0, scale=crstd2[:, cc, :]
                    )
                    nc.vector.scalar_tensor_tensor(
                        out=var,
                        in0=ctq_t,
                        scalar=cmean[:, cc, :],
                        in1=msq,
                        op0=ALU.subtract,
                        op1=ALU.mult,
                    )
                    out_sb = work.tile([128, QB], F32, tag="outb", bufs=1)
                    nc.vector.scalar_tensor_tensor(
                        out=out_sb,
                        in0=mnp,
                        scalar=hb_sb[:, cc, :],
                        in1=var,
                        op0=ALU.add,
                        op1=ALU.add,
                    )
                    nc.sync.dma_start(out=out_ch[:, cc, ts(blk, QB)], in_=out_sb)

    nc.compile()
    return nc


_NC_CACHE = []


def kernel(content, style, content_key, style_key, f_w, f_b, g_w, g_b, h_w, h_b):
    _install_axon_hooks_shim()
    if not _NC_CACHE:
        _NC_CACHE.append(_build())
    nc = _NC_CACHE[0]

    c32 = lambda a: np.ascontiguousarray(np.asarray(a), dtype=np.float32)

    fwT = c32(np.asarray(f_w).T)
    gwT = c32(np.asarray(g_w).T)
    hwT = c32(np.asarray(h_w).T)
    fbr = c32(np.asarray(f_b).reshape(C, 1))
    gbr = c32(np.asarray(g_b).reshape(C, 1))
    hbr = c32(np.asarray(h_b).reshape(C, 1))

    in_maps = []
    for core in range(8):
        b, h = core // 2, core % 2
        qsl = slice(h * Q, (h + 1) * Q)
        in_maps.append(
            {
                "ckq": c32(np.asarray(content_key[b]).reshape(C, HW)[:, qsl]),
                "sk": c32(np.asarray(style_key[b]).reshape(C, HW)),
                "st": c32(np.asarray(style[b]).reshape(C, HW)),
                "ct": c32(np.asarray(content[b]).reshape(C, HW)),
                "ctq": c32(np.asarray(content[b]).reshape(C, HW)[:, qsl]),
                "fwT": fwT,
                "gwT": gwT,
                "hwT": hwT,
                "fb": fbr,
                "gb": gbr,
                "hb": hbr,
            }
        )

    try:
        res = run_bass_kernel_spmd(nc, in_maps, core_ids=list(range(8)), trace=True)
    except Exception:
        res = run_bass_kernel_spmd(nc, in_maps, core_ids=list(range(8)), trace=False)
    kernel.last_exec_time_ns = res.exec_time_ns

    full = np.empty((B, C, HW), dtype=np.float32)
    for core in range(8):
        b, h = core // 2, core % 2
        full[b][:, h * Q : (h + 1) * Q] = res.results[core]["out"]
    return full.reshape(B, C, 64, 64)


kernel.last_exec_time_ns = None
